# revision 37
# baseline (speedup 1.0000x reference)
"""DeepEMD episode loss kernel for Trainium2 (8 NeuronCores, data-parallel over episodes).

Algorithm (per core = one episode):
  - S[q,p,m,n] = cosine-sim over channels of centered features  (PE matmuls, fp32r)
  - entropic-OT via Sinkhorn in scaling form: u = a/(Kv), v = b/(K^T u),
    K = exp((S-1)/eps).  8 iterations match the 100-iter log-domain
    reference to ~1e-8 relative (geometric convergence).
  - logits = T * <S, u K v>;  per-query CE returned, mean taken on host.
"""

import numpy as np
from contextlib import ExitStack

import concourse.bass as bass
import concourse.bacc as bacc
import concourse.tile as tile
from concourse import mybir
from concourse.bass_utils import run_bass_kernel_spmd

F32 = mybir.dt.float32
BF16 = mybir.dt.bfloat16
X = mybir.AxisListType.X
ADD = mybir.AluOpType.add
MULT = mybir.AluOpType.mult
MAX = mybir.AluOpType.max
EXP = mybir.ActivationFunctionType.Exp
LOG = mybir.ActivationFunctionType.Ln

# problem constants (hardcoded per contract)
B = 8          # episodes = cores
Q = 75         # queries
P = 5          # ways (1-shot -> 1 proto per way)
C = 640        # channels
HW = 49        # spatial
QM = Q * HW    # 3675
PN = P * HW    # 245
PNP = 256      # padded moving dim for full-rate fp32r matmul
NT = 25        # partition-groups per way (75 q / 3)
NPART = 125    # 5 ways * 25
J = 3          # pairs per partition
E = HW * HW    # 2401
F = J * E      # 7203
CCH = 128      # contraction chunk
NCC = C // CCH # 5
TEMP = 12.5
EPS = 0.05
ITERS = 5
RSQC = 1.0 / np.sqrt(float(C))
WSCALE = 1.0 / HW
MARG_EPS = float(np.float32(1e-3) + np.float32(1e-5))

QMCH = [(k * 128, min(128, QM - k * 128)) for k in range((QM + 127) // 128)]  # 29
RCH = [(k * 512, min(512, QM - k * 512)) for k in range((QM + 511) // 512)]   # 8


def emit(tc, qry, sup, oh, ce_out, lg_out, gb, wb, qd, w1d, pd, zr):
    nc = tc.nc
    with ExitStack() as ctx:
        rows = ctx.enter_context(tc.tile_pool(name="rows", bufs=1))
        ev = ctx.enter_context(tc.tile_pool(name="ev", bufs=4))
        small = ctx.enter_context(tc.tile_pool(name="small", bufs=1))

        # ---------------- phase A: load inputs ----------------
        with tc.tile_pool(name="big", bufs=1) as big:
            QT = []
            ST = []
            QB = []
            SB = []
            qv = qry.rearrange("q c m -> c q m")
            sv = sup.rearrange("p c n -> c p n")
            for ci in range(NCC):
                # alternate the two HWDGE rings (SP / ACT) for load bandwidth
                dma_eng = nc.sync if ci % 2 == 0 else nc.scalar
                t = big.tile([128, QM], F32, tag=f"qt{ci}")
                dma_eng.dma_start(
                    t[:].rearrange("x (q m) -> x q m", q=Q),
                    qv[ci * CCH:(ci + 1) * CCH],
                )
                QT.append(t)
                s = big.tile([128, PNP], F32, tag=f"st{ci}")
                # zero only the pad columns: keeps the data region single-producer
                # (TensorReduce supports a single sync-wait slot in codegen)
                nc.vector.memset(s[:, PN:], 0.0)
                dma_eng.dma_start(
                    s[:, :PN].rearrange("x (p n) -> x p n", p=P),
                    sv[ci * CCH:(ci + 1) * CCH],
                )
                ST.append(s)
                # bf16 shadows for the PE (G + stats matmuls run in bf16)
                tb = big.tile([128, QM], BF16, tag=f"qb{ci}")
                nc.scalar.copy(tb[:], t[:])
                QB.append(tb)
                sb = big.tile([128, PNP], BF16, tag=f"sb{ci}")
                nc.scalar.copy(sb[:], s[:])
                SB.append(sb)

            # ---------------- phase B: stats + weight rows (PE/ACT/DVE) -------
            # gap sums (over spatial) for weight matmuls
            QG = []
            SG = []
            for ci in range(NCC):
                g = small.tile([128, Q], F32, tag=f"qg{ci}")
                nc.vector.tensor_reduce(
                    g[:], QT[ci][:].rearrange("x (q m) -> x q m", q=Q), axis=X, op=ADD
                )
                QG.append(g)
                h = small.tile([128, P], F32, tag=f"sg{ci}")
                nc.vector.tensor_reduce(
                    h[:], ST[ci][:, :PN].rearrange("x (p n) -> x p n", p=P),
                    axis=X, op=ADD,
                )
                SG.append(h)

            augq = rows.tile([1, QM], F32, tag="augq")
            augqb = rows.tile([1, QM], BF16, tag="augqb")
            ssqq = rows.tile([1, QM], F32, tag="ssqq")
            w1r = rows.tile([P, QM], F32, tag="w1r")
            augp = rows.tile([1, PNP], F32, tag="augp")
            augpb = rows.tile([1, PNP], BF16, tag="augpb")
            ssqp = rows.tile([1, PNP], F32, tag="ssqp")
            w2r = rows.tile([Q, PNP], F32, tag="w2r")

            onesb = small.tile([128, 1], BF16, tag="onesb")
            nc.vector.memset(onesb[:], 1.0)

            # col-sum rows: cmu_q (-> aug rows, bf16 matmul) ; w1 (fp32 matmul)
            with tc.tile_pool(name="ps1", bufs=2, space="PSUM") as ps1:
                for off, wd in RCH:
                    pc = ps1.tile([1, 512], F32, tag="prow")
                    pw = ps1.tile([P, 512], F32, tag="prow_w")
                    for ci in range(NCC):
                        nc.tensor.matmul(pc[:, :wd], onesb[:],
                                         QB[ci][:, off:off + wd],
                                         start=(ci == 0), stop=(ci == NCC - 1))
                        nc.tensor.matmul(pw[:, :wd], SG[ci][:],
                                         QT[ci][:, off:off + wd],
                                         start=(ci == 0), stop=(ci == NCC - 1))
                    nc.scalar.mul(augq[:, off:off + wd], pc[:, :wd], -RSQC)
                    nc.scalar.mul(augqb[:, off:off + wd], pc[:, :wd], -RSQC)
                    nc.scalar.mul(w1r[:, off:off + wd], pw[:, :wd], WSCALE)

            # ssq_q rows (bf16 squares via ACT, one reused tile)
            with tc.tile_pool(name="ps2", bufs=1, space="PSUM") as ps2:
                qsqb = big.tile([128, QM], BF16, tag="qsqb")
                pss = [
                    ps2.tile([1, 512], F32, tag=f"pss{k}", name=f"pss{k}")
                    for k in range(len(RCH))
                ]
                for ci in range(NCC):
                    nc.scalar.square(qsqb[:], QT[ci][:])
                    for k, (off, wd) in enumerate(RCH):
                        nc.tensor.matmul(pss[k][:, :wd], onesb[:],
                                         qsqb[:, off:off + wd],
                                         start=(ci == 0), stop=(ci == NCC - 1))
                for k, (off, wd) in enumerate(RCH):
                    nc.scalar.copy(ssqq[:, off:off + wd], pss[k][:, :wd])

            # sup-side rows: cmu_p, ssq_p (bf16), w2 (fp32)
            with tc.tile_pool(name="ps3", bufs=1, space="PSUM") as ps3:
                pcp = ps3.tile([1, PNP], F32, tag="pcp")
                psp = ps3.tile([1, PNP], F32, tag="psp")
                pw2 = ps3.tile([Q, PNP], F32, tag="pw2")
                ssb5 = ev.tile([128, PNP], BF16, tag="ssb5")
                for ci in range(NCC):
                    nc.tensor.matmul(pcp[:], onesb[:], SB[ci][:],
                                     start=(ci == 0), stop=(ci == NCC - 1))
                    nc.scalar.square(ssb5[:], ST[ci][:])
                    nc.tensor.matmul(psp[:], onesb[:], ssb5[:],
                                     start=(ci == 0), stop=(ci == NCC - 1))
                    nc.tensor.matmul(pw2[:], QG[ci][:], ST[ci][:],
                                     start=(ci == 0), stop=(ci == NCC - 1))
                nc.scalar.mul(augp[:], pcp[:], RSQC)
                nc.scalar.mul(augpb[:], pcp[:], RSQC)
                nc.scalar.copy(ssqp[:], psp[:])
                nc.scalar.mul(w2r[:], pw2[:], WSCALE)
                # bounce w2 through DRAM for the pair-major relayout
                nc.sync.dma_start(wb, w2r[:, :PN])

            # ---------------- phase C: G-hat matmuls (bf16) -> DRAM bounce ---
            with tc.tile_pool(name="ps4", bufs=4, space="PSUM") as ps4:
                for off, wd in QMCH:
                    pg = ps4.tile([128, PNP], F32, tag="pg")
                    for ci in range(NCC):
                        nc.tensor.matmul(
                            pg[:wd], QB[ci][:, off:off + wd], SB[ci][:],
                            start=(ci == 0), stop=False,
                        )
                    nc.tensor.matmul(
                        pg[:wd], augqb[:, off:off + wd], augpb[:],
                        start=False, stop=True,
                    )
                    ge = ev.tile([128, PN], F32, tag="ge")
                    nc.scalar.copy(ge[:wd], pg[:wd, :PN])
                    nc.scalar.dma_start(gb[off:off + wd, :], ge[:wd])

        # ---------------- phase D: pair-major relayouts ------------------
        pair = ctx.enter_context(tc.tile_pool(name="pair", bufs=1))
        # Single-DMA gathers: each pair-layout tile has exactly one producer so
        # consumers never exceed the per-instruction sync-wait limit. The small
        # stat rows bounce through DRAM (qd) for the same reason.
        GP = pair.tile([NPART, F], F32, tag="gp")
        nc.sync.dma_start(
            GP[:].rearrange("x (j m n) -> x j m n", j=J, m=HW),
            gb.rearrange("(t j m) (p n) -> p t j m n", t=NT, j=J, p=P),
        )

        AQP = small.tile([NPART, HW * J], F32, tag="aqp")
        SQP = small.tile([NPART, HW * J], F32, tag="sqp")
        W1P = small.tile([NPART, HW * J], F32, tag="w1p")
        APP = small.tile([NPART, HW * J], F32, tag="app")
        SPP = small.tile([NPART, HW * J], F32, tag="spp")
        W2P = small.tile([NPART, HW * J], F32, tag="w2p")

        # dump the stat rows to DRAM (single producers for the gathers below)
        nc.sync.dma_start(qd[0], augq[:])
        nc.sync.dma_start(qd[1], ssqq[:])
        nc.scalar.dma_start(w1d, w1r[:])
        nc.sync.dma_start(pd[0], augp[:, :PN])
        nc.sync.dma_start(pd[1], ssqp[:, :PN])

        def rep_q(x):  # [QM] -> [p(step0), tjm]  replicated over p
            return x.broadcast_to((QM, P)).rearrange("f p -> p f")

        nc.sync.dma_start(AQP[:], rep_q(qd[0]))
        nc.sync.dma_start(SQP[:], rep_q(qd[1]))
        nc.scalar.dma_start(W1P[:], w1d)

        def rep_p(x):  # [PN] -> [p, tj(step0), n]  broadcast over (t, j)
            return x.rearrange("(p n) -> p n", p=P) \
                    .broadcast_to((P, HW, NT * J)).rearrange("p n t -> p t n")

        nc.sync.dma_start(APP[:].rearrange("x (j n) -> x j n", j=J), rep_p(pd[0]))
        nc.sync.dma_start(SPP[:].rearrange("x (j n) -> x j n", j=J), rep_p(pd[1]))
        nc.scalar.dma_start(
            W2P[:].rearrange("x (j n) -> x j n", j=J),
            wb.rearrange("q (p n) -> p q n", p=P),
        )

        # ---------------- phase E: r-vectors, S, K, SK, marginals --------
        def rsqrt_nr(dstag, aug_t, ssq_t):
            t1 = small.tile([NPART, HW * J], F32, tag="sc1")
            nc.vector.tensor_mul(t1[:], aug_t[:], aug_t[:])
            nsq = small.tile([NPART, HW * J], F32, tag="sc2")
            nc.vector.tensor_sub(nsq[:], ssq_t[:], t1[:])
            nc.vector.tensor_scalar_max(nsq[:], nsq[:], 1e-16)
            sq = small.tile([NPART, HW * J], F32, tag="sc3")
            nc.scalar.sqrt(sq[:], nsq[:])
            y0 = small.tile([NPART, HW * J], F32, tag="sc4")
            nc.vector.reciprocal(y0[:], sq[:])
            # NR: y1 = y0 * (1.5 - 0.5 * nsq * y0^2)
            nc.vector.tensor_mul(t1[:], y0[:], y0[:])
            nc.vector.tensor_mul(t1[:], t1[:], nsq[:])
            nc.vector.tensor_scalar(t1[:], t1[:], -0.5, 1.5, op0=MULT, op1=ADD)
            out = small.tile([NPART, HW * J], F32, tag=dstag)
            nc.vector.tensor_mul(out[:], y0[:], t1[:])
            return out

        RQ = rsqrt_nr("rq", AQP, SQP)
        RP = rsqrt_nr("rp", APP, SPP)

        # S = G * rq (bcast n) * rp (bcast m);  G pair tile is m-major [j][m][n]
        TF = pair.tile([NPART, F], F32, tag="tf")
        nc.vector.tensor_mul(
            TF[:].rearrange("x (j m n) -> x j m n", j=J, m=HW),
            GP[:].rearrange("x (j m n) -> x j m n", j=J, m=HW),
            RQ[:].rearrange("x (j m) -> x j m", j=J).broadcast_to((NPART, J, HW, HW)),
        )
        SP = pair.tile([NPART, F], F32, tag="sp")  # n-major [j][n][m]
        nc.vector.tensor_mul(
            SP[:].rearrange("x (j n m) -> x j n m", j=J, n=HW),
            TF[:].rearrange("x (j m n) -> x j n m", j=J, m=HW),
            RP[:].rearrange("x (j n) -> x j n", j=J).broadcast_to((NPART, J, HW, HW)),
        )
        KK = pair.tile([NPART, F], F32, tag="kk")  # n-major
        bm20 = small.tile([NPART, 1], F32, tag="bm20")
        nc.vector.memset(bm20[:], -1.0 / EPS)
        nc.scalar.activation(KK[:], SP[:], EXP, bias=bm20[:], scale=1.0 / EPS)

        # marginals a (j,m-order), b (j,n-order)
        def marginal(dstag, wsrc):
            wa = small.tile([NPART, HW * J], F32, tag="sc1")
            nc.vector.tensor_scalar(wa[:], wsrc[:], 0.0, MARG_EPS, op0=MAX, op1=ADD)
            sa = small.tile([NPART, J], F32, tag="sc5")
            nc.vector.tensor_reduce(
                sa[:], wa[:].rearrange("x (j m) -> x j m", j=J), axis=X, op=ADD)
            ra = small.tile([NPART, J], F32, tag="sc6")
            nc.vector.reciprocal(ra[:], sa[:])
            out = small.tile([NPART, HW * J], F32, tag=dstag)
            nc.vector.tensor_mul(
                out[:].rearrange("x (j m) -> x j m", j=J),
                wa[:].rearrange("x (j m) -> x j m", j=J),
                ra[:].broadcast_to((NPART, J, HW)),
            )
            return out

        AT = marginal("aa", W1P)
        BT = marginal("bb", W2P)

        # ---------------- phase F: Sinkhorn scaling iterations -----------
        U = small.tile([NPART, HW * J], F32, tag="uu")
        V = small.tile([NPART, HW * J], F32, tag="vv")
        kk_jnm = KK[:].rearrange("x (j n m) -> x j n m", j=J, n=HW)   # natural
        kk_jmn = KK[:].rearrange("x (j n m) -> x j m n", j=J, n=HW)   # transposed view
        rscr = small.tile([NPART, HW * J], F32, tag="rscr")
        for it in range(ITERS):
            su = small.tile([NPART, HW * J], F32, tag="sc1")
            if it == 0:
                # v == 1 -> t = K; reduce K directly
                nc.vector.tensor_reduce(su[:], kk_jmn, axis=X, op=ADD)
            else:
                # t = K * v  (iterate j,n,m; v bcast over m)
                nc.vector.tensor_mul(
                    TF[:].rearrange("x (j n m) -> x j n m", j=J, n=HW),
                    kk_jnm,
                    V[:].rearrange("x (j n) -> x j n", j=J)
                        .broadcast_to((NPART, J, HW, HW)),
                )
                nc.vector.tensor_reduce(
                    su[:], TF[:].rearrange("x (j n m) -> x j m n", j=J, n=HW),
                    axis=X, op=ADD)
            ru = small.tile([NPART, HW * J], F32, tag="sc2")
            nc.vector.reciprocal_approx_fast(ru[:], su[:])
            nc.vector.tensor_mul(U[:], AT[:], ru[:])
            # t2 = K * u (iterate j,m,n; u bcast over n)
            nc.vector.tensor_mul(
                TF[:].rearrange("x (j m n) -> x j m n", j=J, m=HW),
                kk_jmn,
                U[:].rearrange("x (j m) -> x j m", j=J).broadcast_to((NPART, J, HW, HW)),
            )
            sv = small.tile([NPART, HW * J], F32, tag="sc3")
            nc.vector.tensor_reduce(
                sv[:], TF[:].rearrange("x (j m n) -> x j n m", j=J, m=HW),
                axis=X, op=ADD)
            rv = small.tile([NPART, HW * J], F32, tag="sc4")
            if it == ITERS - 1:
                nc.vector.reciprocal_approx_accurate(rv[:], sv[:], rscr[:])
            else:
                nc.vector.reciprocal_approx_fast(rv[:], sv[:])
            nc.vector.tensor_mul(V[:], BT[:], rv[:])

        # ---------------- phase G: logits + CE ---------------------------
        # z = sum_n v_n * sum_m S[j,m,n] * t2[j,m,n], with t2 = K*u from the
        # final iteration still in TF (m-major).
        T3 = pair.tile([NPART, F], F32, tag="gp")  # reuse GP slot
        nc.vector.tensor_mul(
            T3[:].rearrange("x (j m n) -> x j m n", j=J, m=HW),
            TF[:].rearrange("x (j m n) -> x j m n", j=J, m=HW),
            SP[:].rearrange("x (j n m) -> x j m n", j=J, n=HW),
        )
        sm = small.tile([NPART, HW * J], F32, tag="sc1")
        nc.vector.tensor_reduce(
            sm[:], T3[:].rearrange("x (j m n) -> x j n m", j=J, m=HW),
            axis=X, op=ADD)
        t4 = small.tile([NPART, HW * J], F32, tag="sc2")
        nc.vector.tensor_mul(t4[:], sm[:], V[:])
        Z = small.tile([NPART, J], F32, tag="zz")
        nc.vector.tensor_reduce(
            Z[:], t4[:].rearrange("x (j n) -> x j n", j=J), axis=X, op=ADD)

        # Z [(p t), j] -> DRAM -> L [q, p]  (single producer for the CE ops)
        nc.sync.dma_start(zr, Z[:])
        L = small.tile([Q, P], F32, tag="ll")
        nc.sync.dma_start(
            L[:],
            zr.rearrange("(p t) j -> (t j) p", p=P),
        )

        OH = small.tile([Q, P], F32, tag="oh")
        nc.sync.dma_start(OH[:], oh)

        mx = small.tile([Q, 1], F32, tag="mx")
        nc.vector.tensor_reduce(mx[:], L[:], axis=X, op=MAX)
        nmx = small.tile([Q, 1], F32, tag="nmx")
        nc.vector.tensor_scalar_mul(nmx[:], mx[:], -TEMP)
        ee = small.tile([Q, P], F32, tag="ee")
        nc.scalar.activation(ee[:], L[:], EXP, bias=nmx[:], scale=TEMP)
        se = small.tile([Q, 1], F32, tag="se")
        nc.vector.tensor_reduce(se[:], ee[:], axis=X, op=ADD)
        lg = small.tile([Q, 1], F32, tag="lgs")
        zb = small.tile([Q, 1], F32, tag="zb")
        nc.vector.memset(zb[:], 0.0)
        nc.scalar.activation(lg[:], se[:], LOG, bias=zb[:])
        zl5 = small.tile([Q, P], F32, tag="zl5")
        nc.vector.tensor_mul(zl5[:], L[:], OH[:])
        zl = small.tile([Q, 1], F32, tag="zl")
        nc.vector.tensor_reduce(zl[:], zl5[:], axis=X, op=ADD)
        d1 = small.tile([Q, 1], F32, tag="d1")
        nc.vector.tensor_sub(d1[:], mx[:], zl[:])
        ceo = small.tile([Q, 1], F32, tag="ceo")
        nc.vector.scalar_tensor_tensor(ceo[:], d1[:], TEMP, lg[:], op0=MULT, op1=ADD)

        nc.sync.dma_start(ce_out, ceo[:])
        nc.sync.dma_start(lg_out, L[:])


def build_program():
    nc = bacc.Bacc("TRN2", target_bir_lowering=False, debug=False)
    qry = nc.dram_tensor("qry", [Q, C, HW], F32, kind="ExternalInput").ap()
    sup = nc.dram_tensor("sup", [P, C, HW], F32, kind="ExternalInput").ap()
    oh = nc.dram_tensor("oh", [Q, P], F32, kind="ExternalInput").ap()
    ce = nc.dram_tensor("ce", [Q, 1], F32, kind="ExternalOutput").ap()
    lgt = nc.dram_tensor("lgt", [Q, P], F32, kind="ExternalOutput").ap()
    gb = nc.dram_tensor("gb", [QM, PN], F32).ap()
    wb = nc.dram_tensor("wb", [Q, PN], F32).ap()
    qd = nc.dram_tensor("qd", [2, QM], F32).ap()
    w1d = nc.dram_tensor("w1d", [P, QM], F32).ap()
    pd = nc.dram_tensor("pd", [2, PN], F32).ap()
    zr = nc.dram_tensor("zr", [NPART, J], F32).ap()
    with tile.TileContext(nc) as tc:
        emit(tc, qry, sup, oh, ce, lgt, gb, wb, qd, w1d, pd, zr)
    nc.compile()
    return nc


def make_in_maps(support_xf, query_xf, query_y):
    support_xf = np.ascontiguousarray(np.asarray(support_xf, dtype=np.float32))
    query_xf = np.ascontiguousarray(np.asarray(query_xf, dtype=np.float32))
    query_y = np.asarray(query_y)
    in_maps = []
    for i in range(B):
        ohm = np.zeros((Q, P), np.float32)
        ohm[np.arange(Q), query_y[i].astype(np.int64)] = 1.0
        in_maps.append({
            "qry": query_xf[i].reshape(Q, C, HW),
            "sup": support_xf[i].reshape(P, C, HW),
            "oh": ohm,
        })
    return in_maps


def kernel(support_xf, query_xf, support_y, query_y, n_way=5, k_shot=1, **_):
    nc = build_program()
    in_maps = make_in_maps(support_xf, query_xf, query_y)
    res = run_bass_kernel_spmd(nc, in_maps, list(range(B)))
    ce = np.concatenate([res.results[i]["ce"].reshape(-1) for i in range(B)])
    return np.float32(ce.mean())
